# revision 24
# baseline (speedup 1.0000x reference)
"""Trainium2 Bass kernel for nn_CAB (channel-attention block).

8-way batch-parallel (1 sample per NeuronCore). Per core, fused pipeline:
  q/k path in fp8 (errors cancel through l2norm + softmax):
    conv1x1 as fp8 DoubleRow matmuls (K=192 in one pass) ->
    depthwise 3x3 as fp8 DoubleRow diagonal matmuls (2 taps per matmul,
    overlapping-window APs on padded slot buffers) ->
    PE transposes (fp16) -> gram S=q@k^T accumulated in PSUM.
  v path in fp16 (un-normalized, needs precision):
    conv1x1 (PE fp16) -> depthwise 3x3 via fp16 diagonal matmuls.
  Softmax + proj fold (W_effT) exact in fp32; out = W_eff @ v in fp16.

Math identity: with attn A (block-diag per head) and alpha==1, the final 1x1
proj collapses into one matrix W_eff = proj @ A_bd, so out = W_eff @ v.
Per-channel scales applied to q/k conv + dw weights on the host cancel in the
l2 normalization.
"""

import sys

sys.path.insert(0, "/opt/trn_rl_repo")

import numpy as np
import ml_dtypes
from contextlib import ExitStack

import concourse.bass as bass
import concourse.bacc as bacc
import concourse.tile as tile
import concourse.mybir as mybir
from concourse.bass_types import AP
from concourse.bass_utils import run_bass_kernel_spmd

F8 = mybir.dt.float8e4
F16 = mybir.dt.float16
F32 = mybir.dt.float32
E4 = ml_dtypes.float8_e4m3
ALU = mybir.AluOpType
AFT = mybir.ActivationFunctionType
DRMODE = mybir.MatmulPerfMode.DoubleRow

B, C, H, W, HEADS = 8, 192, 128, 128, 8
DH = C // HEADS          # 24
N = H * W                # 16384
MB = 16                  # image rows per megablock
NMB = H // MB            # 8
PADW = W + 2             # 130
SLOTS = MB + 2           # 18 row-slots in padded pre-buffers (halo +-1)
MBF = MB * W             # 2048 free elems per megablock
PBF = SLOTS * PADW       # 2340 flat elems per prebuf
PBTAIL = 4               # extra zero cols so t8 window reads stay in-bounds

# dw tap pairs for DoubleRow: tap t=(dy+1)*3+(dx+1), delta = dy*130+dx.
# Pair strides must be EVEN (odd ifmap pair strides hang the PE), so pair
# same-parity deltas: (t0,t2) (t3,t5) (t6,t8) (t1,t4) (t7,zero).
DW_PAIR_TAPS = [(0, 2), (3, 5), (6, 8), (1, 4), (7, 9)]   # 9 = zero tap
DW_PAIRS = [(-131, 2), (-1, 2), (129, 2), (-130, 130), (130, 2)]
DW_CHUNKS = [(0, 3), (3, 3), (6, 3), (9, 3), (12, 3), (15, 1)]

_CACHE = {}
SKIP = set()  # timing-model knockouts: {"conv","dwdr","dwpe","trans","gram","sq","p3","evict"}


def _dw_cols(w):
    # (ch,1,3,3) -> (ch,9) fp32, tap t=(dy+1)*3+(dx+1)
    return np.ascontiguousarray(w[:, 0].reshape(w.shape[0], 9).astype(np.float32))


def build_nc_fast():
    nc = bacc.Bacc("TRN2", target_bir_lowering=False, debug=False, num_devices=8)

    x8_d = nc.dram_tensor("x8", [96, 2 * N], F8, kind="ExternalInput")
    y8_d = nc.dram_tensor("y8", [96, 2 * N], F8, kind="ExternalInput")
    y_d = nc.dram_tensor("y", [C, N], F16, kind="ExternalInput")
    wq8_d = nc.dram_tensor("wq8", [96, 384], F8, kind="ExternalInput")
    wk8_d = nc.dram_tensor("wk8", [96, 384], F8, kind="ExternalInput")
    wv_d = nc.dram_tensor("wv", [C, C], F16, kind="ExternalInput")   # [cin, cout]
    dwd8_d = nc.dram_tensor("dwd8", [128, 3840], F8, kind="ExternalInput")
    dwdiag_d = nc.dram_tensor("dwdiag", [128, 2880], F16, kind="ExternalInput")
    projr_d = nc.dram_tensor("projr", [C, C], F16, kind="ExternalInput")  # [mid, o]
    miscA_d = nc.dram_tensor("miscA", [96, 8], F32, kind="ExternalInput")
    miscB_d = nc.dram_tensor("miscB", [96, 8], F32, kind="ExternalInput")
    ident_d = nc.dram_tensor("ident", [128, 128], F16, kind="ExternalInput")
    ones_d = nc.dram_tensor("ones96", [1, 96], F32, kind="ExternalInput")
    dmask_d = nc.dram_tensor("dmask", [96, 384], F16, kind="ExternalInput")
    dwv16_d = nc.dram_tensor("dwv16", [128, 9], F32, kind="ExternalInput")
    dwv1p_d = nc.dram_tensor("dwv1p", [128, 384], F16, kind="ExternalInput")
    out_d = nc.dram_tensor("out", [C, N], F16, kind="ExternalOutput")

    with tile.TileContext(nc) as tc, ExitStack() as ctx:
        const = ctx.enter_context(tc.tile_pool(name="const", bufs=1))
        pers = ctx.enter_context(tc.tile_pool(name="pers", bufs=1))
        xio = ctx.enter_context(tc.tile_pool(name="xio", bufs=2))
        stg = ctx.enter_context(tc.tile_pool(name="stg", bufs=2))
        convps = ctx.enter_context(tc.tile_pool(name="convps", bufs=4, space="PSUM"))
        dwps = ctx.enter_context(tc.tile_pool(name="dwps", bufs=2, space="PSUM"))
        gramps = ctx.enter_context(tc.tile_pool(name="gramps", bufs=1, space="PSUM"))
        dwout = ctx.enter_context(tc.tile_pool(name="dwout", bufs=1))
        tsb = ctx.enter_context(tc.tile_pool(name="tsb", bufs=1))
        small = ctx.enter_context(tc.tile_pool(name="small", bufs=1))

        # ---------------- constants into SBUF ----------------
        def cload(name, shape, dt, src_ap):
            t = const.tile(shape, dt, tag=name, name=name)
            nc.sync.dma_start(t[:], src_ap)
            return t

        wq8 = cload("wq8", [96, 384], F8, wq8_d[:, :])
        wk8 = cload("wk8", [96, 384], F8, wk8_d[:, :])
        wv0 = cload("wv0", [128, C], F16, wv_d[0:128, :])
        wv1 = cload("wv1", [64, C], F16, wv_d[128:192, :])
        dwd8 = cload("dwd8", [128, 3840], F8, dwd8_d[:, :])
        dwdiag = cload("dwdiag", [128, 2880], F16, dwdiag_d[:, :])
        projrA = cload("projrA", [96, C], F16, projr_d[0:96, :])
        projrB = cload("projrB", [96, C], F16, projr_d[96:192, :])
        ident = cload("ident", [128, 128], F16, ident_d[:, :])
        ones96 = cload("ones96", [1, 96], F32, ones_d[:, :])
        dmask = cload("dmask", [96, 384], F16, dmask_d[:, :])
        miscA = cload("miscA", [96, 8], F32, miscA_d[:, :])
        miscB = cload("miscB", [96, 8], F32, miscB_d[:, :])
        dwv16 = cload("dwv16", [128, 9], F32, dwv16_d[:, :])
        dwv1p = cload("dwv1p", [128, 384], F16, dwv1p_d[:, :])

        # ---------------- persistent state ----------------
        vres0 = pers.tile([128, N], F16, tag="vres0", name="vres0")
        vres1 = pers.tile([64, N], F16, tag="vres1", name="vres1")

        def prebuf(name, parts, dt):
            bufs = []
            for i in range(2):
                t = pers.tile([parts, PBF + PBTAIL], dt, tag=f"{name}{i}",
                              name=f"{name}{i}")
                pr = t[:, 0:PBF].rearrange("p (r w) -> p r w", w=PADW)
                nc.gpsimd.memset(pr[:, :, 0:1], 0.0)
                nc.gpsimd.memset(pr[:, :, PADW - 1:PADW], 0.0)
                nc.gpsimd.memset(t[:, PBF:PBF + PBTAIL], 0.0)
                bufs.append(t)
            return bufs

        pb_q0 = prebuf("pbq0", 128, F8)
        pb_k0 = prebuf("pbk0", 128, F8)
        pb_q1k1 = prebuf("pbq1k1", 128, F8)
        pb_v0 = prebuf("pbv0", 128, F16)
        pb_v1 = prebuf("pbv1", 128, F16)

        nrm_q0 = pers.tile([128, NMB], F32, tag="nrmq0", name="nrmq0")
        nrm_k0 = pers.tile([128, NMB], F32, tag="nrmk0", name="nrmk0")
        nrm_q1k1 = pers.tile([128, NMB], F32, tag="nrmq1k1", name="nrmq1k1")

        S1a = gramps.tile([96, 96], F32, tag="S1a", name="S1a")
        S1b = gramps.tile([96, 96], F32, tag="S1b", name="S1b")

        # =========== PHASE 1: software-pipelined over megablocks ===========
        def all_pbs():
            return [(pb_q0, 128), (pb_k0, 128), (pb_q1k1, 128), (pb_v0, 128)]

        def slot_ap(pb_t, parts, s0, nrows=4):
            r = pb_t[0:parts, 0:PBF].rearrange("p (r w) -> p r w", w=PADW)
            return r[:, s0:s0 + nrows, 1:1 + W]

        def emit_conv(m):
            bi = m % 2
            n0g = m * MBF
            x8t = xio.tile([96, 2 * MBF], F8, tag="x8t", name="x8t")
            y8t = xio.tile([96, 2 * MBF], F8, tag="y8t", name="y8t")
            x8v = x8_d[:, :].rearrange("p (j n) -> p j n", j=2)
            y8v = y8_d[:, :].rearrange("p (j n) -> p j n", j=2)
            nc.sync.dma_start(
                x8t[:].rearrange("p (j n) -> p j n", j=2),
                x8v[:, :, n0g:n0g + MBF])
            nc.sync.dma_start(
                y8t[:].rearrange("p (j n) -> p j n", j=2),
                y8v[:, :, n0g:n0g + MBF])
            x8m = x8t[:].rearrange("p (j n) -> p j n", j=2)
            y8m = y8t[:].rearrange("p (j n) -> p j n", j=2)
            wq8v = wq8[:].rearrange("p (j m) -> p j m", j=2)
            wk8v = wk8[:].rearrange("p (j m) -> p j m", j=2)
            ya = xio.tile([128, MBF], F16, tag="ya", name="ya")
            yb = xio.tile([64, MBF], F16, tag="yb", name="yb")
            nc.sync.dma_start(ya[:], y_d[0:128, n0g:n0g + MBF])
            nc.sync.dma_start(yb[:], y_d[128:192, n0g:n0g + MBF])

            for sti in range(4):
                nl = sti * 512
                slot0 = sti * 4 + 1

                if "conv" in SKIP:
                    continue
                # q0 via fp8 DoubleRow (full K=192 in one matmul)
                ps_q0 = convps.tile([128, 512], F32, tag="cps", name="ps_q0")
                nc.tensor.matmul(ps_q0[:], wq8v[:, :, 0:128],
                                 x8m[:, :, nl:nl + 512], start=True, stop=True,
                                 perf_mode=DRMODE)
                ps_k0 = convps.tile([128, 512], F32, tag="cps", name="ps_k0")
                nc.tensor.matmul(ps_k0[:], wk8v[:, :, 0:128],
                                 y8m[:, :, nl:nl + 512], start=True, stop=True,
                                 perf_mode=DRMODE)
                ps_q1 = convps.tile([64, 512], F32, tag="cps", name="ps_q1")
                nc.tensor.matmul(ps_q1[:], wq8v[:, :, 128:192],
                                 x8m[:, :, nl:nl + 512], start=True, stop=True,
                                 perf_mode=DRMODE)
                ps_k1 = convps.tile([64, 512], F32, tag="cps", name="ps_k1")
                nc.tensor.matmul(ps_k1[:], wk8v[:, :, 128:192],
                                 y8m[:, :, nl:nl + 512], start=True, stop=True,
                                 perf_mode=DRMODE)
                ps_v0 = convps.tile([128, 512], F32, tag="cps", name="ps_v0")
                nc.tensor.matmul(ps_v0[:], wv0[:, 0:128], ya[:, nl:nl + 512],
                                 start=True, stop=False)
                nc.tensor.matmul(ps_v0[:], wv1[:, 0:128], yb[:, nl:nl + 512],
                                 start=False, stop=True)
                ps_v1 = convps.tile([64, 512], F32, tag="cps", name="ps_v1")
                nc.tensor.matmul(ps_v1[:], wv0[:, 128:192], ya[:, nl:nl + 512],
                                 start=True, stop=False)
                nc.tensor.matmul(ps_v1[:], wv1[:, 128:192], yb[:, nl:nl + 512],
                                 start=False, stop=True)
                if "evict" in SKIP:
                    continue
                nc.scalar.copy(slot_ap(pb_q0[bi], 128, slot0),
                               ps_q0[:].rearrange("p (r w) -> p r w", w=W))
                nc.vector.tensor_copy(slot_ap(pb_k0[bi], 128, slot0),
                                      ps_k0[:].rearrange("p (r w) -> p r w", w=W))
                qk_slots = pb_q1k1[bi][0:128, 0:PBF].rearrange(
                    "p (r w) -> p r w", w=PADW)
                nc.scalar.copy(qk_slots[0:64, slot0:slot0 + 4, 1:1 + W],
                               ps_q1[:].rearrange("p (r w) -> p r w", w=W))
                nc.vector.tensor_copy(qk_slots[64:128, slot0:slot0 + 4, 1:1 + W],
                                      ps_k1[:].rearrange("p (r w) -> p r w", w=W))
                nc.scalar.copy(slot_ap(pb_v0[bi], 128, slot0),
                               ps_v0[:].rearrange("p (r w) -> p r w", w=W))
                v1_slots = pb_v1[bi][0:128, 0:PBF].rearrange(
                    "p (r w) -> p r w", w=PADW)
                nc.scalar.copy(v1_slots[0:64, slot0:slot0 + 4, 1:1 + W],
                               ps_v1[:].rearrange("p (r w) -> p r w", w=W))
                nc.vector.tensor_copy(
                    v1_slots[64:128, slot0 - 1:slot0 + 3, 1:1 + W],
                    ps_v1[:].rearrange("p (r w) -> p r w", w=W))

        def emit_halo(m):
            bi, pi = m % 2, (m - 1) % 2
            for pb, parts in all_pbs():
                is8 = pb[0][:].dtype == F8
                cp = nc.vector.tensor_copy if is8 else nc.gpsimd.tensor_copy
                cur = pb[bi][0:parts, 0:PBF].rearrange("p (r w) -> p r w", w=PADW)
                if m == 0:
                    nc.gpsimd.memset(cur[:, 0:1, :], 0.0)
                else:
                    prev = pb[pi][0:parts, 0:PBF].rearrange("p (r w) -> p r w",
                                                            w=PADW)
                    cp(cur[:, 0:1, :], prev[:, SLOTS - 2:SLOTS - 1, :])
                    cp(prev[:, SLOTS - 1:SLOTS, :], cur[:, 1:2, :])
                if m == NMB - 1:
                    nc.gpsimd.memset(cur[:, SLOTS - 1:SLOTS, :], 0.0)
            # pb_v1 dual-copy: copy1 (0:64, rows at slot r+1) gets the generic
            # halos; copy2 (64:128, rows at slot r) needs slot16 = next row 0
            cur = pb_v1[bi][0:128, 0:PBF].rearrange("p (r w) -> p r w", w=PADW)
            if m == 0:
                nc.gpsimd.memset(cur[0:64, 0:1, :], 0.0)
            else:
                prev = pb_v1[pi][0:128, 0:PBF].rearrange("p (r w) -> p r w",
                                                         w=PADW)
                nc.gpsimd.tensor_copy(cur[0:64, 0:1, :],
                                      prev[0:64, SLOTS - 2:SLOTS - 1, :])
                nc.gpsimd.tensor_copy(prev[0:64, SLOTS - 1:SLOTS, :],
                                      cur[0:64, 1:2, :])
                nc.gpsimd.tensor_copy(prev[64:128, SLOTS - 2:SLOTS - 1, :],
                                      cur[64:128, 0:1, :])
            if m == NMB - 1:
                nc.gpsimd.memset(cur[0:64, SLOTS - 1:SLOTS, :], 0.0)
                nc.gpsimd.memset(cur[64:128, SLOTS - 2:SLOTS - 1, :], 0.0)

        def dw_dr(pb_t, slab, dst_tile):
            # fp8 DoubleRow diag matmuls: 5 tap-pair matmuls per 3-row chunk
            if "dwdr" in SKIP:
                return
            base = pb_t[0:128, :]
            for (r0, nr) in DW_CHUNKS:
                sc = 1 + r0
                free = nr * PADW - 1
                ps = dwps.tile([128, 390], F32, tag="dwps", name="dwps")
                out_ap = ps[:, 1:1 + free]
                for a in range(5):
                    dA, strJ = DW_PAIRS[a]
                    lhsT = dwd8[:, slab * 1280 + a * 256:
                                slab * 1280 + (a + 1) * 256]
                    lhsT = lhsT.rearrange("p (j m) -> p j m", j=2)
                    rhs = AP(base.tensor, base.offset + sc * PADW + dA + 1,
                             [list(base.ap[0]), [strJ, 2], [1, free]])
                    nc.tensor.matmul(out_ap, lhsT, rhs, start=(a == 0),
                                     stop=(a == 4), perf_mode=DRMODE)
                if "evict" in SKIP:
                    continue
                src = ps[:].rearrange("p (r w) -> p r w", w=PADW)[:, 0:nr, 1:129]
                dst = dst_tile[:, r0 * W:(r0 + nr) * W].rearrange(
                    "p (r w) -> p r w", w=W)
                nc.vector.tensor_copy(dst, src)

        def dw_pe(pb_t, parts, diag_off, dst_ap_fn, taps=range(9),
                  merge_in=None):
            # accumulating fp16 diag matmuls per 4-row window on PE; optional
            # fp16 partial (from DVE) merged in via identity matmul
            if "dwpe" in SKIP:
                return
            pr = pb_t[0:parts, 0:PBF].rearrange("p (r w) -> p r w", w=PADW)
            taps = list(taps)
            for gi in range(4):
                ps = convps.tile([128, 512], F32, tag="cps", name="dw_pe_ps")
                for i, t in enumerate(taps):
                    dy, dx = t // 3 - 1, t % 3 - 1
                    s0 = 1 + 4 * gi + dy
                    rhs = pr[:, s0:s0 + 4, 1 + dx:1 + dx + W]
                    nc.tensor.matmul(
                        ps[0:parts, :],
                        dwdiag[0:parts,
                               diag_off + t * parts:diag_off + (t + 1) * parts],
                        rhs, start=(i == 0),
                        stop=(i == len(taps) - 1 and merge_in is None))
                if merge_in is not None:
                    nc.tensor.matmul(ps[0:parts, :], ident[0:parts, 0:parts],
                                     merge_in[0:parts,
                                              gi * 512:(gi + 1) * 512],
                                     start=False, stop=True)
                if gi % 2 == 0:
                    nc.scalar.copy(dst_ap_fn(gi), ps[0:parts, :])
                else:
                    nc.vector.tensor_copy(dst_ap_fn(gi), ps[0:parts, :])

        def dw_tree_taps(pb_t, parts, wcol_tile, wcol_off, taps, dst_tile):
            # DVE partial: sum of w_t * window_t over `taps` (fp16)
            def win(t):
                pr = pb_t[0:parts, 0:PBF].rearrange("p (r w) -> p r w", w=PADW)
                dy, dx = t // 3 - 1, t % 3 - 1
                return pr[:, 1 + dy:1 + dy + MB, 1 + dx:1 + dx + W]

            sA = dst_tile[0:parts, :].rearrange("p (r w) -> p r w", w=W)
            sB_t = dwout.tile([128, MBF], F16, tag="dvescr", name="dvescr")
            sB = sB_t[0:parts, :].rearrange("p (r w) -> p r w", w=W)
            nc.vector.tensor_scalar(sA, win(taps[0]),
                                    wcol_tile[0:parts, wcol_off + taps[0]:
                                              wcol_off + taps[0] + 1],
                                    None, ALU.mult)
            for t in taps[1:]:
                nc.vector.tensor_scalar(sB, win(t),
                                        wcol_tile[0:parts, wcol_off + t:
                                                  wcol_off + t + 1],
                                        None, ALU.mult)
                nc.vector.tensor_tensor(sA, sA, sB, ALU.add)

        def emit_process(m):
            bi = m % 2
            qdw = dwout.tile([128, MBF], F16, tag="qdw", name="qdw")
            kdw = dwout.tile([128, MBF], F16, tag="kdw", name="kdw")
            q1k1dw = dwout.tile([128, MBF], F16, tag="q1k1dw", name="q1k1dw")
            dvp = dwout.tile([128, MBF], F16, tag="dvp", name="dvp")
            dw_dr(pb_q0[bi], 0, qdw)
            if "dwpe" not in SKIP:
                dw_tree_taps(pb_v0[bi], 128, dwv16, 0, [5, 6, 7, 8], dvp)
            dw_dr(pb_k0[bi], 1, kdw)
            dw_dr(pb_q1k1[bi], 2, q1k1dw)
            dw_pe(pb_v0[bi], 128, 0,
                  lambda gi: vres0[:, m * MBF + gi * 512:m * MBF + (gi + 1) * 512],
                  taps=range(5), merge_in=dvp)
            if "dwpe" not in SKIP:
                prv1 = pb_v1[bi][0:128, 0:PBF].rearrange("p (r w) -> p r w",
                                                         w=PADW)
                for gi in range(4):
                    ps = convps.tile([64, 512], F32, tag="cps", name="v1ps")
                    # pairs (t0,t3) (t1,t4) (t2,t5): copy1 reads dy=-1 row,
                    # copy2 (shifted) supplies dy=0 at the same slot offset
                    for i in range(3):
                        dx = i - 1
                        s0 = 4 * gi
                        rhs = prv1[:, s0:s0 + 4, 1 + dx:1 + dx + W]
                        nc.tensor.matmul(
                            ps[:], dwv1p[:, i * 64:(i + 1) * 64], rhs,
                            start=(i == 0), stop=False)
                    # singles t6 t7 t8 (dy=+1) on copy1 half
                    for j in range(3):
                        dx = j - 1
                        s0 = 4 * gi + 2
                        rhs = prv1[0:64, s0:s0 + 4, 1 + dx:1 + dx + W]
                        nc.tensor.matmul(
                            ps[:], dwv1p[0:64, 192 + j * 64:192 + (j + 1) * 64],
                            rhs, start=False, stop=(j == 2))
                    dst = vres1[:, m * MBF + gi * 512:m * MBF + (gi + 1) * 512]
                    if gi % 2 == 0:
                        nc.scalar.copy(dst, ps[:])
                    else:
                        nc.vector.tensor_copy(dst, ps[:])

            # norms (sum of squares per channel) on ACT
            def sq_accum(src_ap, dst_col):
                scr = dwout.tile([128, MBF], F16, tag="sqscr", name="sqscr")
                nc.scalar.activation(scr[:], src_ap, AFT.Square,
                                     accum_out=dst_col)
            if "sq" not in SKIP:
                sq_accum(qdw[:], nrm_q0[:, m:m + 1])
                sq_accum(kdw[:], nrm_k0[:, m:m + 1])
                sq_accum(q1k1dw[:], nrm_q1k1[:, m:m + 1])

            # transposes via DMA XBAR + gram accumulation
            if "trans" in SKIP:
                return
            qT = tsb.tile([128, 16 * 192], F16, tag="qT", name="qT")
            kT = tsb.tile([128, 16 * 192], F16, tag="kT", name="kT")
            qTv = qT[:].rearrange("p (b c) -> p b c", c=192)
            kTv = kT[:].rearrange("p (b c) -> p b c", c=192)
            nc.sync.dma_start(qTv[:, :, 0:128], qdw[:], transpose=True)
            nc.sync.dma_start(qTv[:, :, 128:192], q1k1dw[0:64, :],
                              transpose=True)
            nc.sync.dma_start(kTv[:, :, 0:128], kdw[:], transpose=True)
            nc.sync.dma_start(kTv[:, :, 128:192], q1k1dw[64:128, :],
                              transpose=True)
            for b in range(16):
                row = m * MB + b
                st = row == 0
                sp = row == H - 1
                nc.tensor.matmul(S1a[:], qTv[:, b, 0:96], kTv[:, b, 0:96],
                                 start=st, stop=sp)
                nc.tensor.matmul(S1b[:], qTv[:, b, 96:192], kTv[:, b, 96:192],
                                 start=st, stop=sp)

        for m in range(NMB):
            emit_conv(m)
            emit_halo(m)
            if m >= 1:
                emit_process(m - 1)
        emit_process(NMB - 1)

        # =========== PHASE 2: softmax + W_eff fold (small) ===========
        Ssb1 = small.tile([96, 192], F32, tag="Ssb1", name="Ssb1")
        nc.scalar.copy(Ssb1[:, 0:96], S1a[:])
        nc.scalar.copy(Ssb1[:, 96:192], S1b[:])
        nq0 = small.tile([128, 1], F32, tag="nq0", name="nq0")
        nk0 = small.tile([128, 1], F32, tag="nk0", name="nk0")
        nq1k1 = small.tile([128, 1], F32, tag="nq1k1", name="nq1k1")
        nc.vector.tensor_reduce(nq0[:], nrm_q0[:], mybir.AxisListType.X, ALU.add)
        nc.vector.tensor_reduce(nk0[:], nrm_k0[:], mybir.AxisListType.X, ALU.add)
        nc.vector.tensor_reduce(nq1k1[:], nrm_q1k1[:], mybir.AxisListType.X,
                                ALU.add)

        _rs_ctr = [0]

        def rsqrt_col(dst, src_ap, parts):
            _rs_ctr[0] += 1
            t = small.tile([128, 1], F32, tag=f"rs{_rs_ctr[0]}",
                           name=f"rs{_rs_ctr[0]}")
            nc.scalar.sqrt(t[0:parts, :], src_ap)
            nc.vector.tensor_scalar_max(t[0:parts, :], t[0:parts, :], 1e-12)
            nc.vector.reciprocal(dst, t[0:parts, :])
            return dst

        rqa = small.tile([96, 1], F32, tag="rqa", name="rqa")
        rqb = small.tile([96, 1], F32, tag="rqb", name="rqb")
        nqb = small.tile([96, 1], F32, tag="nqb", name="nqb")
        nc.sync.dma_start(nqb[0:32, :], nq0[96:128, :])
        nc.sync.dma_start(nqb[32:96, :], nq1k1[0:64, :])
        rsqrt_col(rqa[:], nq0[0:96, :], 96)
        rsqrt_col(rqb[:], nqb[:], 96)
        nc.vector.tensor_tensor(rqa[:], rqa[:], miscA[:, 0:1], ALU.mult)
        nc.vector.tensor_tensor(rqb[:], rqb[:], miscB[:, 0:1], ALU.mult)

        nk1 = small.tile([64, 1], F32, tag="nk1", name="nk1")
        nc.sync.dma_start(nk1[:], nq1k1[64:128, :])
        nk0h = small.tile([128, 1], F16, tag="nk0h", name="nk0h")
        nk1h = small.tile([64, 1], F16, tag="nk1h", name="nk1h")
        nc.scalar.copy(nk0h[:], nk0[:])
        nc.scalar.copy(nk1h[:], nk1[:])
        rk_ps = gramps.tile([1, 192], F16, tag="S1a", name="rk_ps")
        nc.tensor.transpose(rk_ps[:, 0:128], nk0h[:], ident[:, :])
        nc.tensor.transpose(rk_ps[:, 128:192], nk1h[:], ident[0:64, 0:64])
        rk_row = small.tile([1, 192], F32, tag="rkrow", name="rk_row")
        nc.scalar.sqrt(rk_row[:], rk_ps[:])
        nc.vector.tensor_scalar_max(rk_row[:], rk_row[:], 1e-12)
        nc.vector.reciprocal(rk_row[:], rk_row[:])
        rkb_ps = gramps.tile([96, 192], F32, tag="S1b", name="rkb_ps")
        nc.tensor.matmul(rkb_ps[:], ones96[:], rk_row[:], start=True, stop=True)
        rkb = small.tile([96, 192], F32, tag="rkb", name="rkb")
        nc.scalar.copy(rkb[:], rkb_ps[:])

        def softmax_block(Ssb, rqa_c, rqb_c, tag):
            # Ssb [96,192]: cols 0:96 = (q rows 0:96) x (k 0:96);
            # cols 96:192 = (q rows 96:192) x (k 96:192)
            for half, rq_c in ((0, rqa_c), (96, rqb_c)):
                h = Ssb[:, half:half + 96]
                nc.vector.tensor_tensor(h, h, rkb[:, half:half + 96], ALU.mult)
                nc.scalar.mul(h, h, rq_c)
            ex = small.tile([96, 192], F32, tag=f"ex_{tag}", name=f"ex_{tag}")
            nc.scalar.activation(ex[:], Ssb[:], AFT.Exp)
            sums = small.tile([96, 8], F32, tag=f"sums_{tag}",
                              name=f"sums_{tag}")
            nc.vector.tensor_reduce(
                sums[:], ex[:].rearrange("p (h j) -> p h j", j=DH),
                mybir.AxisListType.X, ALU.add)
            nc.vector.reciprocal(sums[:], sums[:])
            A = small.tile([96, 192], F32, tag=f"A_{tag}", name=f"A_{tag}")
            for blk in range(8):
                nc.vector.tensor_scalar_mul(
                    A[:, blk * DH:(blk + 1) * DH], ex[:, blk * DH:(blk + 1) * DH],
                    sums[:, blk:blk + 1])
            return A

        A1 = softmax_block(Ssb1, rqa[:], rqb[:], "a1")

        M1a = small.tile([96, 96], F16, tag="M1a", name="M1a")
        M1b = small.tile([96, 96], F16, tag="M1b", name="M1b")
        nc.vector.tensor_tensor(M1a[:], A1[:, 0:96], dmask[:, 0:96], ALU.mult)
        nc.vector.tensor_tensor(M1b[:], A1[:, 96:192], dmask[:, 96:192],
                                ALU.mult)

        # W_effT[i, o]: i 0:96 from M1a (mids 0:96), i 96:192 from M1b
        # (mids 96:192); three matmuls with disjoint dst partition ranges
        WeT_psA = convps.tile([96, 192], F32, tag="cps", name="WeT_psA")
        WeT_psB = convps.tile([32, 192], F32, tag="cps", name="WeT_psB")
        WeT_ps1 = convps.tile([64, 192], F32, tag="cps", name="WeT_ps1")
        nc.tensor.matmul(WeT_psA[:], M1a[:], projrA[:], start=True, stop=True)
        nc.tensor.matmul(WeT_psB[:], M1b[:, 0:32], projrB[:],
                         start=True, stop=True)
        nc.tensor.matmul(WeT_ps1[:], M1b[:, 32:96], projrB[:],
                         start=True, stop=True)
        WeT0 = small.tile([128, 192], F16, tag="WeT0", name="WeT0")
        WeT1 = small.tile([64, 192], F16, tag="WeT1", name="WeT1")
        nc.scalar.copy(WeT0[0:96, :], WeT_psA[:])
        nc.vector.tensor_copy(WeT0[96:128, :], WeT_psB[:])
        nc.scalar.copy(WeT1[:], WeT_ps1[:])

        # =========== PHASE 3: out = W_eff @ v ===========
        evictors = [nc.scalar.copy, nc.vector.tensor_copy]
        for mb3 in range(0 if "p3" in SKIP else NMB):
            ob = stg.tile([128, MBF], F16, tag="ob", name="ob")
            os_ = stg.tile([64, MBF], F16, tag="os", name="os")
            for t4 in range(4):
                t = mb3 * 4 + t4
                sl = slice(t * 512, (t + 1) * 512)
                lsl = slice(t4 * 512, (t4 + 1) * 512)
                big = convps.tile([128, 512], F32, tag="cps", name="p3_big")
                sm = convps.tile([64, 512], F32, tag="cps", name="p3_sm")
                nc.tensor.matmul(big[:], WeT0[:, 0:128], vres0[:, sl],
                                 start=True, stop=False)
                nc.tensor.matmul(big[:], WeT1[:, 0:128], vres1[:, sl],
                                 start=False, stop=True)
                nc.tensor.matmul(sm[:], WeT0[:, 128:192], vres0[:, sl],
                                 start=True, stop=False)
                nc.tensor.matmul(sm[:], WeT1[:, 128:192], vres1[:, sl],
                                 start=False, stop=True)
                evictors[t4 % 2](ob[:, lsl], big[:])
                evictors[(t4 + 1) % 2](os_[:, lsl], sm[:])
            nc.sync.dma_start(out_d[0:128, mb3 * MBF:(mb3 + 1) * MBF], ob[:])
            nc.sync.dma_start(out_d[128:192, mb3 * MBF:(mb3 + 1) * MBF], os_[:])

    nc.compile()
    return nc


def _rowscale(w):
    s = np.abs(w).max(axis=1, keepdims=True)
    s = np.where(s == 0, 1.0, s)
    return w / s


def _prep_fast(inputs):
    x = np.asarray(inputs["x"], np.float32)
    y = np.asarray(inputs["y"], np.float32)
    q_w = np.asarray(inputs["q_w"], np.float32)[:, :, 0, 0]      # [out,in]
    kv_w = np.asarray(inputs["kv_w"], np.float32)[:, :, 0, 0]    # [2C,in]
    proj_w = np.asarray(inputs["proj_w"], np.float32)[:, :, 0, 0]
    q_dw = _dw_cols(np.asarray(inputs["q_dw_w"], np.float32))
    kv_dw = _dw_cols(np.asarray(inputs["kv_dw_w"], np.float32))
    temp1 = np.asarray(inputs["temp1"], np.float32).reshape(HEADS)
    temp2 = np.asarray(inputs["temp2"], np.float32).reshape(HEADS)
    alpha = np.asarray(inputs["alpha"], np.float32).reshape(C)

    k_dw, v_dw = kv_dw[0:C], kv_dw[C:2 * C]

    # fp8 conv weights for q,k: [in, out] layout, per-out-channel rowscaled
    # (scale cancels in l2norm)
    Wq_s = _rowscale(q_w)
    Wk_s = _rowscale(kv_w[0:C])
    wq8 = np.ascontiguousarray(
        Wq_s.T.reshape(96, 2, C)).astype(E4).reshape(96, 384)
    wk8 = np.ascontiguousarray(
        Wk_s.T.reshape(96, 2, C)).astype(E4).reshape(96, 384)

    # fp8 dw diag tables: 3 slabs (qA, kA, qkB), 5 tap-pairs each.
    q_dw_s = _rowscale(q_dw)
    k_dw_s = _rowscale(k_dw)
    dwd8 = np.zeros((128, 3, 5, 2, 128), dtype=np.float32)
    slabs = [q_dw_s[0:128], k_dw_s[0:128],
             np.concatenate([q_dw_s[128:192], k_dw_s[128:192]], 0)]
    for s, wt in enumerate(slabs):
        for a in range(5):
            for j in range(2):
                t = DW_PAIR_TAPS[a][j]
                if t > 8:
                    continue
                np.fill_diagonal(dwd8[:, s, a, j, :], wt[:, t])
    dwd8 = dwd8.astype(E4).reshape(128, 3840)

    # fp16 diag tables for v dw (baseline layout: v0 at 0, v1 at 2304)
    dwdiag = np.zeros((128, 2880), np.float16)
    for t in range(9):
        np.fill_diagonal(dwdiag[:, t * 128:(t + 1) * 128], v_dw[0:128, t])
        np.fill_diagonal(dwdiag[0:64, 2304 + t * 64:2304 + (t + 1) * 64],
                         v_dw[128:192, t])

    dmask = np.zeros((96, 384), np.float16)
    for h in range(4):
        dmask[h * DH:(h + 1) * DH, h * DH:(h + 1) * DH] = 1.0
        dmask[h * DH:(h + 1) * DH, 96 + h * DH:96 + (h + 1) * DH] = 1.0

    tempq = np.repeat(temp1, DH)
    misc = np.zeros((C, 8), np.float32)
    misc[:, 0] = tempq
    misc[:, 1] = np.repeat(temp2, DH)
    misc[:, 2] = alpha
    misc[:, 3] = 1.0 - alpha

    dwv16 = np.zeros((128, 9), np.float32)
    dwv16[:, :] = v_dw[0:128]
    v1w = v_dw[128:192]                          # [64, 9]
    dwv1p = np.zeros((128, 384), np.float16)
    for i, (ta, tb) in enumerate(((0, 3), (1, 4), (2, 5))):
        np.fill_diagonal(dwv1p[0:64, i * 64:(i + 1) * 64], v1w[:, ta])
        np.fill_diagonal(dwv1p[64:128, i * 64:(i + 1) * 64], v1w[:, tb])
    for j, t in enumerate((6, 7, 8)):
        np.fill_diagonal(dwv1p[0:64, 192 + j * 64:192 + (j + 1) * 64],
                         v1w[:, t])

    shared = {
        "wq8": wq8,
        "wk8": wk8,
        "dwv16": dwv16,
        "dwv1p": dwv1p,
        "wv": np.ascontiguousarray(kv_w[C:2 * C].T.astype(np.float16)),
        "dwd8": dwd8,
        "dwdiag": dwdiag,
        "projr": np.ascontiguousarray(proj_w.T.astype(np.float16)),
        "miscA": np.ascontiguousarray(misc[0:96]),
        "miscB": np.ascontiguousarray(misc[96:192]),
        "ident": np.eye(128, dtype=np.float16),
        "ones96": np.ones((1, 96), np.float32),
        "dmask": dmask,
    }
    in_maps = []
    for i in range(B):
        im = dict(shared)
        xi = x[i].reshape(C, N)
        yi = y[i].reshape(C, N)
        im["x8"] = np.ascontiguousarray(xi).astype(E4).reshape(96, 2 * N)
        im["y8"] = np.ascontiguousarray(yi).astype(E4).reshape(96, 2 * N)
        im["y"] = np.ascontiguousarray(yi.astype(np.float16))
        in_maps.append(im)
    return in_maps


def kernel(**inputs) -> np.ndarray:
    alpha = np.asarray(inputs["alpha"], np.float32).reshape(C)
    full_path = not np.all(alpha == 1.0)
    if full_path:
        import kernel_baseline
        return kernel_baseline.kernel(**inputs)
    in_maps = _prep_fast(inputs)
    if "fast" not in _CACHE:
        _CACHE["fast"] = build_nc_fast()
    nc = _CACHE["fast"]
    res = run_bass_kernel_spmd(nc, in_maps, list(range(B)))
    out = np.stack([np.asarray(res.results[i]["out"]).astype(np.float32)
                    .reshape(C, H, W) for i in range(B)])
    return out


if __name__ == "__main__":
    import reference
    inputs = reference.setup_inputs()
    expected = np.asarray(reference.reference(**inputs))
    actual = kernel(**{k: np.asarray(v) for k, v in inputs.items()})
    err = np.abs(actual - expected).max() / (np.abs(expected).max() + 1e-30)
    print("Relative error:", err)
